# revision 1
# baseline (speedup 1.0000x reference)
import numpy as np
import jax
import jax.numpy as jnp

# Hardcoded problem shapes (nn_GCNEmbedding2): data [32,64,8,256], adj [32,64,64]
B, N, CIN, L, FM = 32, 64, 8, 256, 64
EPS = 1e-5
NDEV = 8
BLOC = B // NDEV  # graphs per core


def _conv1d(x, w, b):
    k = w.shape[-1]
    out = jax.lax.conv_general_dilated(
        x, w, (1,), [(k // 2, k // 2)], dimension_numbers=("NCH", "OIH", "NCH"))
    return out + b[None, :, None]


def _bn(x, gamma, beta):
    C, Lx = x.shape[1], x.shape[2]
    xr = x.reshape(BLOC, N, C, Lx)
    mean = xr.mean(axis=(1, 3), keepdims=True)
    var = xr.var(axis=(1, 3), keepdims=True)
    xn = (xr - mean) * jax.lax.rsqrt(var + EPS)
    xn = xn * gamma[None, None, :, None] + beta[None, None, :, None]
    return xn.reshape(BLOC * N, C, Lx)


def _block(x, p, expand):
    out = jax.nn.relu(_bn(_conv1d(x, p["wx"], p["bx"]), p["gx"], p["betax"]))
    out = jax.nn.relu(_bn(_conv1d(out, p["wy"], p["by"]), p["gy"], p["betay"]))
    out = _bn(_conv1d(out, p["wz"], p["bz"]), p["gz"], p["betaz"])
    sc = _conv1d(x, p["ws"], p["bs"]) if expand else x
    sc = _bn(sc, p["gs"], p["betas"])
    return jax.nn.relu(out + sc)


def _fwd(data, adj, params):
    x = data.reshape(BLOC * N, CIN, L)
    for i in range(4):
        x = _block(x, params["block%d" % (i + 1)], expand=(i == 0))
    x = x.mean(axis=-1).reshape(BLOC, N, FM)
    support = jnp.einsum("bnf,fo->bno", x, params["gc_w"])
    out = jnp.einsum("bnm,bmo->bno", adj, support) + params["gc_b"]
    return out.mean(axis=1)


_pfwd = jax.pmap(_fwd, in_axes=(0, 0, None))


def kernel(data, adj, params, idx):
    idx = np.asarray(idx)
    d = np.asarray(data)[idx].reshape(NDEV, BLOC, N, CIN, L)
    a = np.asarray(adj)[idx].reshape(NDEV, BLOC, N, N)
    params = jax.tree_util.tree_map(np.asarray, params)
    out = _pfwd(d, a, params)
    return np.asarray(out).reshape(B, FM).astype(np.float32)
